# revision 55
# baseline (speedup 1.0000x reference)
"""Trainium2 Bass kernel for the gnn_message_passing block (8 NeuronCores).

Strategy (per core c, owning 512 global rows r = c*512..(c+1)*512):
  - Host rotates x_node/x_edge rows by -r0 so the owned rows sit first on
    every core (SPMD: one program, per-core data).
  - Associativity: mat @ (x @ W.T) == (mat @ x) @ W.T, so the five big
    N x N aggregations (adj@h shared by modules 0/4, four proj@k inputs)
    are computed ONCE per core as row-blocks in fp8(e4m3) DoubleRow
    matmuls (2x PE rate; mats pre-scaled host-side to O(1), descale
    folded into Wq/Wk), producing feature-major f16 outputs that feed
    the 512x512 projections. The aggregates only perturb softmax
    logits, so fp8 error (~4% rms) costs ~1e-2 final rel err at most.
  - rmsnorm weight vectors and the 1/sqrt(D) score scale are folded into
    the projection weights host-side; on-chip rmsnorm is the pure
    x * rsqrt(mean(x^2)+eps) form, computed on ACT (square+accum).
  - Per-node 8-head SDPA runs on DVE in fp16 (2x mode) with broadcast-AP
    multiplies and halving-tree + segmented reduces; exp on ACT.
  - Module processing is interleaved with the aggregations so PE (matmuls)
    and DVE (SDPA) run concurrently:
      loads(h,e0-3) | mod 1,5 | agg0 | mod 0,4 | agg1,2 | loads(e) |
      agg3 | mod 3 | agg4 | mod 7 | mod 2 | FFN-h | mod 6 | FFN-e
  - FFNs: feature-major matmuls, gelu(+bias) on ACT, bias2 via K=1 matmul.
Projection/FFN matmuls use float32r (full-rate fp32, moving dim >= 256).
DMA loads are batched (4-8 tiles per SWDGE start) to keep Pool free.
"""
import numpy as np

N = 4096
E = 512
H = 8
D = 64
FF = 2048
P = 128
NCORES = 8
RPC = N // NCORES  # 512 rows per core
NT = N // P        # 32 tiles over all nodes
LT = RPC // P      # 4 local tiles
EPS = float(np.finfo(np.float32).eps)

_PROGRAM_CACHE = {}


def _split_big_waits(nc, mybir):
    """walrus in this toolchain rejects multi-wait instructions; cap at 1
    (2 for EventSemaphore), chaining the excess as EventSemaphores."""
    for f in nc.m.functions:
        for bb in f.blocks:
            insts = list(bb.instructions)
            out = []
            changed = False
            for inst in insts:
                si = inst.sync_info
                waits = list(si.on_wait) if si and si.on_wait else []
                cap = 2 if isinstance(inst, mybir.InstEventSemaphore) else 1
                if len(waits) > cap:
                    extra, keep = waits[:-cap], waits[-cap:]
                    for ci in range(0, len(extra), 2):
                        ev = mybir.InstEventSemaphore(name=f"{inst.name}-evw{ci}")
                        ev.engine = inst.engine
                        ev.sync_info = mybir.SyncInfo(on_wait=extra[ci:ci + 2],
                                                      on_update=[])
                        out.append(ev)
                    si.on_wait = keep
                    changed = True
                out.append(inst)
            if changed:
                bb.instructions[:] = out


def _build_program():
    import concourse.bass as bass
    import concourse.tile as tile
    from concourse import mybir
    from concourse.masks import make_identity
    from contextlib import ExitStack

    f32 = mybir.dt.float32
    f32r = mybir.dt.float32r
    f16 = mybir.dt.float16
    bf16 = mybir.dt.bfloat16
    f8 = mybir.dt.float8e4
    AF = mybir.ActivationFunctionType
    OP = mybir.AluOpType
    AX = mybir.AxisListType
    DR = mybir.MatmulPerfMode.DoubleRow

    def bc(t, dims, off=0):
        return bass.AP(tensor=t.tensor, offset=t.offset + off,
                       ap=[list(t.ap[0])] + [[s, c] for (s, c) in dims])

    nc = bass.Bass()

    xn_d = nc.declare_dram_parameter("xn", [N, E], f16, isOutput=False)
    xe_d = nc.declare_dram_parameter("xe", [N, E], f16, isOutput=False)
    # host-permuted: [group g, partition p, pair t, plane two, row r] so each
    # partition reads one contiguous 4KB line per group DMA
    mat_d = [nc.declare_dram_parameter(f"mat{i}", [4, P, 4, 2, RPC], f8,
                                       isOutput=False)
             for i in range(5)]
    # merged qkv: [module, 3(q/k/v), partition p, fc, e] -> one DMA per module
    wqkv_d = nc.declare_dram_parameter("wqkv", [H, 3, P, 4, E], f16,
                                       isOutput=False)
    w1h_d = nc.declare_dram_parameter("w1hT", [E, FF], f16, isOutput=False)
    w2h_d = nc.declare_dram_parameter("w2hT", [FF, E], f16, isOutput=False)
    w1e_d = nc.declare_dram_parameter("w1eT", [E, FF], f16, isOutput=False)
    w2e_d = nc.declare_dram_parameter("w2eT", [FF, E], f16, isOutput=False)
    b1h_d = nc.declare_dram_parameter("b1h", [FF], f32, isOutput=False)
    b2h_d = nc.declare_dram_parameter("b2h", [E], f32, isOutput=False)
    b1e_d = nc.declare_dram_parameter("b1e", [FF], f32, isOutput=False)
    b2e_d = nc.declare_dram_parameter("b2e", [E], f32, isOutput=False)
    outh_d = nc.declare_dram_parameter("outh", [RPC, E], f32, isOutput=True)
    oute_d = nc.declare_dram_parameter("oute", [RPC, E], f32, isOutput=True)

    with tile.TileContext(nc, pool_alloc_mode="queue") as tc, ExitStack() as ctx:
        consts = ctx.enter_context(tc.tile_pool(name="consts", bufs=1))
        ident = consts.tile([P, P], f32)
        make_identity(nc, ident)
        ones1f = consts.tile([1, P], f32)
        nc.gpsimd.memset(ones1f, 1.0)
        ones1 = consts.tile([1, P], f32r)
        nc.scalar.copy(ones1[:], ones1f[:])
        eps_t = consts.tile([P, 1], f32)
        nc.vector.memset(eps_t, EPS)
        b1h_t = consts.tile([P, FF // P], f32)
        nc.sync.dma_start(out=b1h_t, in_=b1h_d[:].rearrange("(c p) -> p c", p=P))
        b1e_t = consts.tile([P, FF // P], f32)
        nc.sync.dma_start(out=b1e_t, in_=b1e_d[:].rearrange("(c p) -> p c", p=P))
        b2h_t = consts.tile([1, E], f32r)
        nc.gpsimd.dma_start(out=b2h_t, in_=b2h_d[:].rearrange("(a e) -> a e", a=1))
        b2e_t = consts.tile([1, E], f32r)
        nc.gpsimd.dma_start(out=b2e_t, in_=b2e_d[:].rearrange("(a e) -> a e", a=1))

        # whole-program pools
        locp = ctx.enter_context(tc.tile_pool(name="loc", bufs=1))
        attp = ctx.enter_context(tc.tile_pool(name="att", bufs=1))
        statp = ctx.enter_context(tc.tile_pool(name="stat", bufs=4))
        sqscp = ctx.enter_context(tc.tile_pool(name="sqsc", bufs=1))
        wpool = ctx.enter_context(tc.tile_pool(name="wts", bufs=1))
        qkvp = ctx.enter_context(tc.tile_pool(name="qkv", bufs=1))
        tmpp = ctx.enter_context(tc.tile_pool(name="sdtmp", bufs=1))
        smp = ctx.enter_context(tc.tile_pool(name="sdsm", bufs=2))
        psp = ctx.enter_context(tc.tile_pool(name="ps", bufs=1, space="PSUM"))

        hTl = [locp.tile([P, RPC], f16, tag=f"hTl{fc}", name=f"hTl{fc}")
               for fc in range(4)]
        eTl = [locp.tile([P, RPC], f16, tag=f"eTl{fc}", name=f"eTl{fc}")
               for fc in range(4)]
        xatt_h = [attp.tile([P, E], f32, tag=f"xh{t}", name=f"xh{t}")
                  for t in range(LT)]
        xatt_e = [attp.tile([P, E], f32, tag=f"xe{t}", name=f"xe{t}")
                  for t in range(LT)]

        def rmsnorm_tile(dst_ap, src_ap, dst2=None, mul_eng=None):
            """dst = pure rmsnorm of node-major [128, 512] slice."""
            sc = sqscp.tile([P, E], f32, tag="sqsc", name="sqsc")
            ssq = statp.tile([P, 1], f32, tag="ssq", name="ssq")
            nc.scalar.activation(out=sc[:], in_=src_ap, func=AF.Square,
                                 accum_out=ssq[:])
            sq = statp.tile([P, 1], f32, tag="sq", name="sq")
            nc.scalar.activation(out=sq[:], in_=ssq[:], func=AF.Sqrt,
                                 bias=eps_t[:], scale=1.0 / E)
            rs = statp.tile([P, 1], f32, tag="rs", name="rs")
            nc.vector.reciprocal(out=rs[:], in_=sq[:])
            if mul_eng == "act":
                nc.scalar.activation(out=dst_ap, in_=src_ap, func=AF.Copy,
                                     scale=rs[:])
            else:
                nc.gpsimd.tensor_scalar_mul(dst_ap, src_ap, rs[:])
            if dst2 is not None:
                nc.gpsimd.tensor_scalar_mul(dst2[:], src_ap, rs[:])

        def load_norm(x_dram, dst_pairs, t0, t1, xpool, dst2=None, qoff=0):
            """Stream x rows [t0*128, t1*128) in 4-tile DMAs; rmsnorm each
            into fp8 pair-tile halves (dst_pairs[ti//2], half ti%2).
            Bulk groups alternate between the HWDGE (sync) and SWDGE
            (Pool) queues so x never serializes behind the mat stream."""
            for gi, g0 in enumerate(range(t0, t1, 4)):
                xg = xpool.tile([P, 4 * E], f16, tag="xing", name="xing")
                eng = nc.sync if (gi + qoff) % 2 == 0 else nc.gpsimd
                eng.dma_start(
                    out=xg.rearrange("p (t e) -> p t e", e=E),
                    in_=x_dram[g0 * P:(g0 + 4) * P, :].rearrange(
                        "(t p) e -> p t e", p=P))
                for t in range(4):
                    ti = g0 + t
                    pr = dst_pairs[ti // 2]
                    half = (ti % 2) * E
                    rmsnorm_tile(pr[:, half:half + E],
                                 xg[:, t * E:(t + 1) * E],
                                 dst2=(dst2[ti] if dst2 and ti < 4 else None))

        def transpose_local(srcs, dstT):
            for fc in range(4):
                ps = psp.tile([P, RPC], f32, tag="projps", bufs=4, name="trps")
                for t in range(4):
                    nc.tensor.transpose(ps[:, t * P:(t + 1) * P],
                                        srcs[t][:, fc * P:(fc + 1) * P]
                                        .bitcast(f32),
                                        ident[:])
                nc.scalar.copy(dstT[fc][:], ps[:])

        def aggregate(mi, lhs_pairs, aggpool, scale=1.0):
            """returns 4 feature-major f16 [128, 512] blocks of mat_mi @ x.

            fp8 DoubleRow: contraction over pairs of 128-node planes; lhsT
            free dims (2, 128 feats), rhs free dims (2, 512 rows)."""
            mst = ExitStack()
            matgp = mst.enter_context(tc.tile_pool(name=f"matg{mi}", bufs=2))
            pss = [psp.tile([P, E], f32, tag=f"agps{b}", name=f"agps{b}")
                   for b in range(4)]
            NPAIR = NT // 2
            for g in range(4):
                mt = matgp.tile([P, 4 * 2 * RPC], f8, tag="matg", name="matg")
                nc.sync.dma_start(
                    out=mt.rearrange("p (t two e) -> p t two e", e=RPC, two=2),
                    in_=mat_d[mi][g])
                for t in range(4):
                    pi = g * 4 + t
                    for b in range(4):
                        nc.tensor.matmul(
                            pss[b][:],
                            lhsT=bc(lhs_pairs[pi], [(E, 2), (1, P)], off=b * P),
                            rhs=bc(mt, [(RPC, 2), (1, RPC)], off=t * 2 * RPC),
                            start=(pi == 0), stop=(pi == NPAIR - 1),
                            perf_mode=DR)
            outt = []
            for b in range(4):
                at = aggpool.tile([P, E], f16, tag=f"ag{mi}_{b}",
                                  name=f"ag{mi}_{b}")
                nc.scalar.activation(out=at[:], in_=pss[b][:], func=AF.Copy,
                                     scale=scale)
                outt.append(at)
            mst.close()
            return outt

        def wload(m):
            """One merged q|k|v weight DMA for module m (12KB/partition).
            Issued on the Pool SWDGE queue so weights never queue behind
            the bulk x/mat stream on HWDGE."""
            wt = wpool.tile([P, 3 * 4 * E], f16, tag="wqkv", bufs=3,
                            name=f"wqkv{m}")
            nc.gpsimd.dma_start(
                out=wt.rearrange("p (k fc e) -> p k fc e", e=E, fc=4),
                in_=wqkv_d[m])
            return wt

        def module_proj(m, qsrc, ksrc, wt=None):
            if wt is None:
                wt = wload(m)
            q_sb, k_sb, v_sb = [], [], []
            for (srcT, ki, lst) in ((qsrc, 0, q_sb), (ksrc, 1, k_sb),
                                    (hTl, 2, v_sb)):
                for b in range(LT):
                    ps = psp.tile([P, E], f32, tag="projps", bufs=4,
                                  name="projps")
                    for fc in range(4):
                        nc.tensor.matmul(
                            ps[:],
                            lhsT=srcT[fc][:, b * P:(b + 1) * P],
                            rhs=wt[:, (ki * 4 + fc) * E:(ki * 4 + fc + 1) * E],
                            start=(fc == 0), stop=(fc == 3))
                    dt = qkvp.tile([P, E], f16, tag=f"qkv{ki}_{b}", bufs=2,
                                   name=f"qkv{b}")
                    if ki == 2:
                        nc.scalar.copy(bc(dt, [(1, 8), (8, 64)]), ps[:])
                    else:
                        nc.scalar.copy(dt[:], ps[:])
                    lst.append(dt)
            return (q_sb, k_sb, v_sb)

        def module_sdpa(qkv, t, branch_att, first, offload=True):
                q_sb, k_sb, v_sb = qkv
                q_t, k_t, v_t = q_sb[t], k_sb[t], v_sb[t]
                tmp = tmpp.tile([P, H * H * D], f16, tag="sdpa", bufs=4,
                                name="sdpa")
                nc.vector.tensor_tensor(
                    out=bc(tmp, [(512, 8), (64, 8), (1, 64)]),
                    in0=bc(q_t, [(64, 8), (0, 8), (1, 64)]),
                    in1=bc(k_t, [(0, 8), (64, 8), (1, 64)]),
                    op=OP.mult)
                # first halving level on Pool for paired modules (Pool has
                # slack; DVE is critical). Solo modules keep the chain on
                # DVE to avoid cross-engine ping-pong latency.
                eng0 = nc.gpsimd if offload else nc.vector
                eng0.tensor_tensor(
                    out=bc(tmp, [(64, 64), (1, 32)]),
                    in0=bc(tmp, [(64, 64), (1, 32)]),
                    in1=bc(tmp, [(64, 64), (1, 32)], off=32),
                    op=OP.add)
                for dd in (16, 8, 4, 2):
                    nc.vector.tensor_tensor(
                        out=bc(tmp, [(64, 64), (1, dd)]),
                        in0=bc(tmp, [(64, 64), (1, dd)]),
                        in1=bc(tmp, [(64, 64), (1, dd)], off=dd),
                        op=OP.add)
                s_t = smp.tile([P, H * H], f32, tag="s", name="s")
                nc.vector.tensor_reduce(
                    out=s_t[:], in_=bc(tmp, [(64, 64), (1, 2)]),
                    axis=AX.X, op=OP.add)
                ex_t = smp.tile([P, H * H], f16, tag="ex", name="ex")
                nc.scalar.activation(out=ex_t[:], in_=s_t[:], func=AF.Exp)
                den = smp.tile([P, H], f32, tag="den", name="den")
                nc.vector.tensor_reduce(
                    out=den[:], in_=ex_t.rearrange("p (h g) -> p h g", g=H),
                    axis=AX.X, op=OP.add)
                rden = smp.tile([P, H], f32, tag="rden", name="rden")
                nc.vector.reciprocal(out=rden[:], in_=den[:])
                a_t = smp.tile([P, H * H], f16, tag="a", name="a")
                nc.vector.tensor_tensor(
                    out=bc(a_t, [(8, 8), (1, 8)]),
                    in0=bc(ex_t, [(8, 8), (1, 8)]),
                    in1=bc(rden, [(1, 8), (0, 8)]),
                    op=OP.mult)
                tmp2 = tmpp.tile([P, H * H * D], f16, tag="sdpa", bufs=4,
                                 name="sdpa2")
                nc.vector.tensor_tensor(
                    out=bc(tmp2, [(512, 8), (8, 64), (1, 8)]),
                    in0=bc(a_t, [(8, 8), (0, 64), (1, 8)]),
                    in1=bc(v_t, [(0, 8), (8, 64), (1, 8)]),
                    op=OP.mult)
                for gg in (4, 2):
                    nc.vector.tensor_tensor(
                        out=bc(tmp2, [(8, 512), (1, gg)]),
                        in0=bc(tmp2, [(8, 512), (1, gg)]),
                        in1=bc(tmp2, [(8, 512), (1, gg)], off=gg),
                        op=OP.add)
                if first:
                    nc.vector.tensor_tensor(
                        out=branch_att[t][:],
                        in0=bc(tmp2, [(8, 512)]),
                        in1=bc(tmp2, [(8, 512)], off=1),
                        op=OP.add)
                else:
                    # pair-sum + accumulate both on Pool (strided 1x ops;
                    # frees DVE for the next tile's products)
                    rt = smp.tile([P, E], f32, tag="avred", name="avred")
                    nc.gpsimd.tensor_tensor(
                        out=rt[:],
                        in0=bc(tmp2, [(8, 512)]),
                        in1=bc(tmp2, [(8, 512)], off=1),
                        op=OP.add)
                    nc.gpsimd.tensor_tensor(out=branch_att[t][:],
                                            in0=branch_att[t][:], in1=rt[:],
                                            op=OP.add)

        def module(m, qsrc, ksrc, branch_att, first, wt=None, offload=True):
            qkv = module_proj(m, qsrc, ksrc, wt)
            for t in range(LT):
                module_sdpa(qkv, t, branch_att, first, offload=offload)

        def ffn_load(w1_dram, w2_dram, b1_t):
            """FFN weight DMAs on the Pool SWDGE queue."""
            st = {"stack": ExitStack(), "b1": b1_t, "xn": [None] * LT}
            fsb = st["stack"].enter_context(tc.tile_pool(name="ffn_sb", bufs=1))
            w2_t = fsb.tile([P, 16 * E], f16, tag="w2", name="w2")
            nc.gpsimd.dma_start(
                out=w2_t.rearrange("p (fc e) -> p fc e", e=E),
                in_=w2_dram[:, :].rearrange("(fc p) e -> p fc e", p=P))
            w1_t = fsb.tile([P, 4 * FF], f16, tag="w1", name="w1")
            nc.gpsimd.dma_start(
                out=w1_t.rearrange("p (fc e) -> p fc e", e=FF),
                in_=w1_dram[:, :].rearrange("(fc p) e -> p fc e", p=P))
            st.update(fsb=fsb, w1=w1_t, w2=w2_t)
            return st

        def ffn_norm(st, branch_att, t):
            xt = st["fsb"].tile([P, E], f32, tag=f"fx{t}", name=f"fx{t}")
            rmsnorm_tile(xt[:], branch_att[t][:])
            st["xn"][t] = xt

        def ffn_tr(st):
            """Transpose normed tiles feature-major; pin layer-2 psums."""
            xn_tiles, xnT = st["xn"], []
            for fc in range(4):
                ps = psp.tile([P, RPC], f32, tag="projps", bufs=4,
                              name="ftr")
                for t in range(4):
                    nc.tensor.transpose(ps[:, t * P:(t + 1) * P],
                                        xn_tiles[t][:, fc * P:(fc + 1) * P],
                                        ident[:])
                xt = st["fsb"].tile([P, RPC], f16, tag=f"fxT{fc}",
                                    name=f"fxT{fc}")
                nc.scalar.copy(xt[:], ps[:])
                xnT.append(xt)
            st.update(xnT=xnT,
                      pss2=[psp.tile([P, E], f32, tag=f"agps{b}",
                                     name=f"fo{b}") for b in range(LT)])

        def ffn_start(branch_att, w1_dram, w2_dram, b1_t):
            st = ffn_load(w1_dram, w2_dram, b1_t)
            for t in range(LT):
                ffn_norm(st, branch_att, t)
            ffn_tr(st)
            return st

        def ffn_blocks(st, ffb0, ffb1):
            """Hidden blocks [ffb0, ffb1): layer1 -> gelu -> layer2 accum."""
            w1_t, w2_t, xnT, pss2 = st["w1"], st["w2"], st["xnT"], st["pss2"]
            for ffb in range(ffb0, ffb1):
                ps = psp.tile([P, RPC], f32, tag="projps", bufs=4,
                              name="fps1")
                for fc in range(4):
                    nc.tensor.matmul(
                        ps[:],
                        lhsT=w1_t[:, fc * FF + ffb * P:fc * FF + (ffb + 1) * P],
                        rhs=xnT[fc][:],
                        start=(fc == 0), stop=(fc == 3))
                gt = st["fsb"].tile([P, RPC], f16, tag=f"g1_{ffb % 4}",
                                    name=f"g1_{ffb}")
                nc.scalar.activation(out=gt[:], in_=ps[:], func=AF.Gelu,
                                     bias=st["b1"][:, ffb:ffb + 1], scale=1.0)
                for b in range(LT):
                    nc.tensor.matmul(
                        pss2[b][:],
                        lhsT=gt[:, b * P:(b + 1) * P],
                        rhs=w2_t[:, ffb * E:(ffb + 1) * E],
                        start=(ffb == 0), stop=False)

        def ffn_finish(st, b2_t, out_dram):
            pss2 = st["pss2"]
            ot = st["fsb"].tile([P, 4 * E], f32, tag="fo", name="fo")
            for b in range(LT):
                nc.tensor.matmul(pss2[b][:], lhsT=ones1[:], rhs=b2_t[:],
                                 start=False, stop=True)
                nc.scalar.copy(ot[:, b * E:(b + 1) * E], pss2[b][:])
                nc.sync.dma_start(
                    out=out_dram[b * P:(b + 1) * P, :],
                    in_=ot[:, b * E:(b + 1) * E])
            st["stack"].close()

        def ffn(branch_att, w1_dram, w2_dram, b1_t, b2_t, out_dram):
            st = ffn_start(branch_att, w1_dram, w2_dram, b1_t)
            ffn_blocks(st, 0, FF // P)
            ffn_finish(st, b2_t, out_dram)

        # ======== emission order (the schedule) ========
        # Pool open/close must be LIFO: agglate (aggs 1-4) and eearly
        # outlive hfull; agg0/erest nest inside.
        aggl12_stack = ExitStack()
        agglate12 = aggl12_stack.enter_context(
            tc.tile_pool(name="agglate12", bufs=1))
        aggl34_stack = ExitStack()
        agglate34 = aggl34_stack.enter_context(
            tc.tile_pool(name="agglate34", bufs=1))
        eearly_stack = ExitStack()
        eearly = eearly_stack.enter_context(tc.tile_pool(name="eearly", bufs=1))
        hstack = ExitStack()
        hfp = hstack.enter_context(tc.tile_pool(name="hfull", bufs=1))

        h_sb = [hfp.tile([P, 2 * E], f8, tag=f"h{t}", name=f"hsb{t}")
                for t in range(NT // 2)]
        e_sb = [eearly.tile([P, 2 * E], f8, tag=f"e{t}", name=f"esb{t}")
                for t in range(2)]
        x04_stack = ExitStack()
        x04p = x04_stack.enter_context(tc.tile_pool(name="x04", bufs=1))
        h04 = [x04p.tile([P, E], f32r, tag=f"h04_{t}", name=f"h04_{t}")
               for t in range(4)]
        e04 = [x04p.tile([P, E], f32r, tag=f"e04_{t}", name=f"e04_{t}")
               for t in range(4)]

        # weights for the two agg-free modules go first in the DMA queue
        wt1 = wload(1)
        wt5 = wload(5)
        load_norm(xe_d, e_sb, 0, 4, eearly, dst2=e04)
        load_norm(xn_d, h_sb, 0, 4, hfp, dst2=h04)
        transpose_local(e04, eTl)
        transpose_local(h04, hTl)
        x04_stack.close()

        # modules 1 and 5 need no aggregates (only hTl/eTl) - start DVE early
        module(1, eTl, eTl, xatt_h, first=True, wt=wt1, offload=False)
        module(5, eTl, eTl, xatt_e, first=True, wt=wt5, offload=False)

        load_norm(xn_d, h_sb, 4, NT, hfp)

        # interleave aggregate and projection emission so the PE stream
        # always runs ahead of the DVE (SDPA) stream
        agg0_stack = ExitStack()
        agg0pool = agg0_stack.enter_context(tc.tile_pool(name="agg0p", bufs=1))
        agg0 = aggregate(0, h_sb, agg0pool, scale=1.0 / 512)
        qkv0 = module_proj(0, agg0, hTl)
        agg1 = aggregate(1, h_sb, agglate12)
        for t in range(LT):
            module_sdpa(qkv0, t, xatt_h, first=False)
        qkv4 = module_proj(4, agg0, hTl)
        agg2 = aggregate(2, h_sb, agglate12)
        for t in range(LT):
            module_sdpa(qkv4, t, xatt_e, first=False)
        agg0_stack.close()
        hstack.close()

        qkv2 = module_proj(2, eTl, agg1)

        # rest of e
        erest_stack = ExitStack()
        erest = erest_stack.enter_context(tc.tile_pool(name="erest", bufs=1))
        e_sb += [erest.tile([P, 2 * E], f8, tag=f"e{t}", name=f"esb{t}")
                 for t in range(2, NT // 2)]
        load_norm(xe_d, e_sb, 4, NT, erest)
        for t in range(LT):
            module_sdpa(qkv2, t, xatt_h, first=False, offload=False)

        agg3 = aggregate(3, e_sb, agglate34)
        qkv3 = module_proj(3, hTl, agg3)
        agg4 = aggregate(4, e_sb, agglate34)
        for t in range(LT):
            module_sdpa(qkv3, t, xatt_h, first=False, offload=False)
        erest_stack.close()
        eearly_stack.close()

        # ---- tail: interleave ffn-h's PE blocks with m7/m6 SDPA (DVE) so
        # neither engine runs solo (PE executes in emission order).
        qkv7 = module_proj(7, hTl, agg4)
        fh = ffn_start(xatt_h, w1h_d, w2h_d, b1h_t)
        module_sdpa(qkv7, 0, xatt_e, False)
        ffn_blocks(fh, 0, 3)
        module_sdpa(qkv7, 1, xatt_e, False)
        ffn_blocks(fh, 3, 6)
        qkv6 = module_proj(6, eTl, agg2)
        module_sdpa(qkv7, 2, xatt_e, False)
        ffn_blocks(fh, 6, 9)
        module_sdpa(qkv7, 3, xatt_e, False)
        ffn_blocks(fh, 9, 11)
        module_sdpa(qkv6, 0, xatt_e, False)
        ffn_blocks(fh, 11, 13)
        module_sdpa(qkv6, 1, xatt_e, False)
        ffn_blocks(fh, 13, 16)
        ffn_finish(fh, b2h_t, outh_d)
        fe = ffn_load(w1e_d, w2e_d, b1e_t)
        module_sdpa(qkv6, 2, xatt_e, False)
        ffn_norm(fe, xatt_e, 0)
        ffn_norm(fe, xatt_e, 1)
        module_sdpa(qkv6, 3, xatt_e, False)
        ffn_norm(fe, xatt_e, 2)
        ffn_norm(fe, xatt_e, 3)
        ffn_tr(fe)
        ffn_blocks(fe, 0, FF // P)
        ffn_finish(fe, b2e_t, oute_d)
        aggl34_stack.close()
        aggl12_stack.close()

    _split_big_waits(nc, mybir)
    return nc


def _get_program():
    if "nc" not in _PROGRAM_CACHE:
        _PROGRAM_CACHE["nc"] = _build_program()
    return _PROGRAM_CACHE["nc"]


def _prep_inputs(x_node, x_edge, adj, Wq, Wk, Wv,
                 proj_he_h, proj_eh_h, proj_he_e, proj_eh_e,
                 rms1_h, rms1_e, rms2_h,
                 w1_h, b1_h, w2_h, b2_h, w1_e, b1_e, w2_e, b2_e):
    """Per-core input dicts. Weight folding + row rotation happen here.

    Aggregation mats are stored fp8(e4m3), pre-scaled so entries are O(1):
    adj x4096 (entries U[0, 1/4096]) and proj x8 (entries N(0, 1/64)).
    The descale is folded back into the downstream projection weights
    (q for modules 0/4 via agg0; k for modules 2/3/6/7 via aggs 1-4);
    for adj, 1/512 of the 1/4096 descale lives in the on-chip psum->f16
    copy (scale arg) so the folded f16 weights stay in normal range."""
    import ml_dtypes
    f = np.float32
    h16 = np.float16
    f8 = ml_dtypes.float8_e4m3
    wsrc_q = [rms1_h, rms1_e, rms1_e, rms1_h, rms1_h, rms1_e, rms1_e, rms1_h]
    wsrc_k = [rms1_h, rms1_e, rms1_h, rms1_e, rms1_h, rms1_e, rms1_h, rms1_e]
    qde = [1 / 8., 1, 1, 1, 1 / 8., 1, 1, 1]   # rest of adj descale (x 1/512 on-chip)
    kde = [1, 1, 1 / 8., 1 / 8., 1, 1, 1 / 8., 1 / 8.]  # proj descale
    wqT = np.stack([(Wq[m].T * wsrc_q[m][:, None]) * (0.125 * qde[m])
                    for m in range(H)])
    wkT = np.stack([Wk[m].T * wsrc_k[m][:, None] * kde[m] for m in range(H)])
    wvT = np.stack([Wv[m].T * rms1_h[:, None] for m in range(H)])
    # merged [H, p, 3(q/k/v), fc, e]: one contiguous 12KB line per partition
    wqkv = np.stack([wqT, wkT, wvT], axis=1).reshape(H, 3, 4, 128, E)
    wqkv = np.ascontiguousarray(wqkv.transpose(0, 3, 1, 2, 4), h16)
    w1hT = np.ascontiguousarray((w1_h * rms2_h[None, :]).T, dtype=h16)
    w1eT = np.ascontiguousarray((w1_e * rms2_h[None, :]).T, dtype=h16)
    w2hT = np.ascontiguousarray(w2_h.T, dtype=h16)
    w2eT = np.ascontiguousarray(w2_e.T, dtype=h16)
    mats = [adj * 4096., proj_eh_h * 8., proj_eh_e * 8.,
            proj_he_h * 8., proj_he_e * 8.]

    shared = dict(wqkv=wqkv,
                  w1hT=w1hT, w2hT=w2hT, w1eT=w1eT, w2eT=w2eT,
                  b1h=b1_h.astype(f), b2h=b2_h.astype(f),
                  b1e=b1_e.astype(f), b2e=b2_e.astype(f))
    in_maps = []
    for c in range(NCORES):
        r0 = c * RPC
        m = dict(shared)
        m["xn"] = np.ascontiguousarray(np.roll(x_node, -r0, axis=0), h16)
        m["xe"] = np.ascontiguousarray(np.roll(x_edge, -r0, axis=0), h16)
        for i, mat in enumerate(mats):
            mt = np.roll(np.ascontiguousarray(mat[r0:r0 + RPC].T),
                         -r0, axis=0).astype(f8)  # [N, RPC] node-major
            # [g, p, t, two, r]: node n = ((g*4 + t)*2 + two)*128 + p
            m[f"mat{i}"] = np.ascontiguousarray(
                mt.reshape(4, 4, 2, 128, RPC).transpose(0, 3, 1, 2, 4))
        in_maps.append(m)
    return in_maps


def kernel(**inputs):
    from concourse.bass_utils import run_bass_kernel_spmd
    nc = _get_program()
    in_maps = _prep_inputs(**{k: np.asarray(v) for k, v in inputs.items()})
    res = run_bass_kernel_spmd(nc, in_maps, list(range(NCORES))).results
    x_h = np.concatenate([res[c]["outh"] for c in range(NCORES)], axis=0)
    x_e = np.concatenate([res[c]["oute"] for c in range(NCORES)], axis=0)
    return (x_h, x_e)



# revision 64
# speedup vs baseline: 3.0406x; 3.0406x over previous
"""Trainium2 Bass kernel for the gnn_message_passing block (8 NeuronCores).

Strategy (per core c, owning 512 global rows r = c*512..(c+1)*512):
  - Host rotates x_node/x_edge rows by -r0 so the owned rows sit first on
    every core (SPMD: one program, per-core data).
  - Associativity: mat @ (x @ W.T) == (mat @ x) @ W.T, so the five big
    N x N aggregations (adj@h shared by modules 0/4, four proj@k inputs)
    are computed ONCE per core as row-blocks in fp8(e4m3) DoubleRow
    matmuls (2x PE rate; mats pre-scaled host-side to O(1), descale
    folded into Wq/Wk), producing feature-major f16 outputs that feed
    the 512x512 projections. The aggregates only perturb softmax
    logits, so fp8 error (~4% rms) costs ~1e-2 final rel err at most.
  - rmsnorm weight vectors and the 1/sqrt(D) score scale are folded into
    the projection weights host-side; on-chip rmsnorm is the pure
    x * rsqrt(mean(x^2)+eps) form, computed on ACT (square+accum).
  - Per-node 8-head SDPA runs on DVE in fp16 (2x mode) with broadcast-AP
    multiplies and halving trees; exp on ACT; first tree level and the
    final accumulate are offloaded to Pool when Pool has phase slack.
  - All x/w/mat DRAM storage is 16-bit or fp8 (host-side cast) and laid
    out so each partition reads 2-12KB contiguous lines; bulk streams
    alternate between the HWDGE (sync) and SWDGE (Pool) DMA queues.
  - Emission interleaves aggregate, projection, SDPA, and FFN-block
    chunks so the in-order PE stream always runs ahead of DVE; the
    h-branch FFN blocks are woven between module 7/6 SDPA tiles, and
    FFN layer-2 accumulation is pipelined per hidden block:
      loads(e,h 0-3) | mod 1,5 | agg0 | mod0 | agg1 | mod4 | agg2 |
      mod2 | loads(e-rest) | agg3 | mod3 | agg4 |
      [proj7 | FFN-h start | sdpa7 x FFN-h blocks | proj6 | sdpa6] |
      FFN-h finish | FFN-e
  - FFNs: feature-major f16 matmuls, gelu(+bias) on ACT, bias2 via K=1
    matmul.
"""
import numpy as np

N = 4096
E = 512
H = 8
D = 64
FF = 2048
P = 128
NCORES = 8
RPC = N // NCORES  # 512 rows per core
NT = N // P        # 32 tiles over all nodes
LT = RPC // P      # 4 local tiles
EPS = float(np.finfo(np.float32).eps)

_PROGRAM_CACHE = {}


def _split_big_waits(nc, mybir):
    """walrus in this toolchain rejects multi-wait instructions; cap at 1
    (2 for EventSemaphore), chaining the excess as EventSemaphores."""
    for f in nc.m.functions:
        for bb in f.blocks:
            insts = list(bb.instructions)
            out = []
            changed = False
            for inst in insts:
                si = inst.sync_info
                waits = list(si.on_wait) if si and si.on_wait else []
                cap = 2 if isinstance(inst, mybir.InstEventSemaphore) else 1
                if len(waits) > cap:
                    extra, keep = waits[:-cap], waits[-cap:]
                    for ci in range(0, len(extra), 2):
                        ev = mybir.InstEventSemaphore(name=f"{inst.name}-evw{ci}")
                        ev.engine = inst.engine
                        ev.sync_info = mybir.SyncInfo(on_wait=extra[ci:ci + 2],
                                                      on_update=[])
                        out.append(ev)
                    si.on_wait = keep
                    changed = True
                out.append(inst)
            if changed:
                bb.instructions[:] = out


def _build_program():
    import concourse.bass as bass
    import concourse.tile as tile
    from concourse import mybir
    from concourse.masks import make_identity
    from contextlib import ExitStack

    f32 = mybir.dt.float32
    f32r = mybir.dt.float32r
    f16 = mybir.dt.float16
    f8 = mybir.dt.float8e4
    AF = mybir.ActivationFunctionType
    OP = mybir.AluOpType
    AX = mybir.AxisListType
    DR = mybir.MatmulPerfMode.DoubleRow

    def bc(t, dims, off=0):
        return bass.AP(tensor=t.tensor, offset=t.offset + off,
                       ap=[list(t.ap[0])] + [[s, c] for (s, c) in dims])

    nc = bass.Bass()

    xn_d = nc.declare_dram_parameter("xn", [N, E], f16, isOutput=False)
    xe_d = nc.declare_dram_parameter("xe", [N, E], f16, isOutput=False)
    # host-permuted: [group g, partition p, pair t, plane two, row r] so each
    # partition reads one contiguous 4KB line per group DMA
    mat_d = [nc.declare_dram_parameter(f"mat{i}", [4, P, 4, 2, RPC], f8,
                                       isOutput=False)
             for i in range(5)]
    # merged qkv: [module, 3(q/k/v), partition p, fc, e] -> one DMA per module
    wqkv_d = nc.declare_dram_parameter("wqkv", [H, 3, P, 4, E], f16,
                                       isOutput=False)
    w1h_d = nc.declare_dram_parameter("w1hT", [E, FF], f16, isOutput=False)
    w2h_d = nc.declare_dram_parameter("w2hT", [FF, E], f16, isOutput=False)
    w1e_d = nc.declare_dram_parameter("w1eT", [E, FF], f16, isOutput=False)
    w2e_d = nc.declare_dram_parameter("w2eT", [FF, E], f16, isOutput=False)
    b1h_d = nc.declare_dram_parameter("b1h", [FF], f32, isOutput=False)
    b2h_d = nc.declare_dram_parameter("b2h", [E], f32, isOutput=False)
    b1e_d = nc.declare_dram_parameter("b1e", [FF], f32, isOutput=False)
    b2e_d = nc.declare_dram_parameter("b2e", [E], f32, isOutput=False)
    outh_d = nc.declare_dram_parameter("outh", [RPC, E], f32, isOutput=True)
    oute_d = nc.declare_dram_parameter("oute", [RPC, E], f32, isOutput=True)

    with tile.TileContext(nc, pool_alloc_mode="queue") as tc, ExitStack() as ctx:
        consts = ctx.enter_context(tc.tile_pool(name="consts", bufs=1))
        ident = consts.tile([P, P], f32)
        make_identity(nc, ident)
        ones1f = consts.tile([1, P], f32)
        nc.gpsimd.memset(ones1f, 1.0)
        ones1 = consts.tile([1, P], f32r)
        nc.scalar.copy(ones1[:], ones1f[:])
        eps_t = consts.tile([P, 1], f32)
        nc.vector.memset(eps_t, EPS)
        b1h_t = consts.tile([P, FF // P], f32)
        nc.sync.dma_start(out=b1h_t, in_=b1h_d[:].rearrange("(c p) -> p c", p=P))
        b1e_t = consts.tile([P, FF // P], f32)
        nc.sync.dma_start(out=b1e_t, in_=b1e_d[:].rearrange("(c p) -> p c", p=P))
        b2h_t = consts.tile([1, E], f32r)
        nc.gpsimd.dma_start(out=b2h_t, in_=b2h_d[:].rearrange("(a e) -> a e", a=1))
        b2e_t = consts.tile([1, E], f32r)
        nc.gpsimd.dma_start(out=b2e_t, in_=b2e_d[:].rearrange("(a e) -> a e", a=1))

        # whole-program pools
        locp = ctx.enter_context(tc.tile_pool(name="loc", bufs=1))
        attp = ctx.enter_context(tc.tile_pool(name="att", bufs=1))
        statp = ctx.enter_context(tc.tile_pool(name="stat", bufs=4))
        sqscp = ctx.enter_context(tc.tile_pool(name="sqsc", bufs=1))
        wpool = ctx.enter_context(tc.tile_pool(name="wts", bufs=1))
        qkvp = ctx.enter_context(tc.tile_pool(name="qkv", bufs=1))
        tmpp = ctx.enter_context(tc.tile_pool(name="sdtmp", bufs=1))
        smp = ctx.enter_context(tc.tile_pool(name="sdsm", bufs=4))
        psp = ctx.enter_context(tc.tile_pool(name="ps", bufs=1, space="PSUM"))

        hTl = [locp.tile([P, RPC], f16, tag=f"hTl{fc}", name=f"hTl{fc}")
               for fc in range(4)]
        eTl = [locp.tile([P, RPC], f16, tag=f"eTl{fc}", name=f"eTl{fc}")
               for fc in range(4)]
        xatt_h = [attp.tile([P, E], f32, tag=f"xh{t}", name=f"xh{t}")
                  for t in range(LT)]
        xatt_e = [attp.tile([P, E], f32, tag=f"xe{t}", name=f"xe{t}")
                  for t in range(LT)]

        def rmsnorm_tile(dst_ap, src_ap, dst2=None, mul_eng=None):
            """dst = pure rmsnorm of node-major [128, 512] slice."""
            sc = sqscp.tile([P, E], f32, tag="sqsc", name="sqsc")
            ssq = statp.tile([P, 1], f32, tag="ssq", name="ssq")
            nc.scalar.activation(out=sc[:], in_=src_ap, func=AF.Square,
                                 accum_out=ssq[:])
            sq = statp.tile([P, 1], f32, tag="sq", name="sq")
            nc.scalar.activation(out=sq[:], in_=ssq[:], func=AF.Sqrt,
                                 bias=eps_t[:], scale=1.0 / E)
            rs = statp.tile([P, 1], f32, tag="rs", name="rs")
            nc.vector.reciprocal(out=rs[:], in_=sq[:])
            if mul_eng == "act":
                nc.scalar.activation(out=dst_ap, in_=src_ap, func=AF.Copy,
                                     scale=rs[:])
            else:
                nc.gpsimd.tensor_scalar_mul(dst_ap, src_ap, rs[:])
            if dst2 is not None:
                nc.gpsimd.tensor_scalar_mul(dst2[:], src_ap, rs[:])

        def load_norm(x_dram, dst_pairs, t0, t1, xpool, dst2=None, qoff=0,
                      mul_eng=None, pertile=False):
            """Stream x rows [t0*128, t1*128) in 4-tile DMAs; rmsnorm each
            into fp8 pair-tile halves (dst_pairs[ti//2], half ti%2).
            Bulk groups alternate between the HWDGE (sync) and SWDGE
            (Pool) queues so x never serializes behind the mat stream."""
            if pertile:
                # startup path: single-tile DMAs so rmsnorm begins ASAP
                for t in range(t0, t1):
                    xg = xpool.tile([P, E], f16, tag="xing1", name="xing1")
                    eng = nc.sync if (t + qoff) % 2 == 0 else nc.gpsimd
                    eng.dma_start(out=xg[:], in_=x_dram[t * P:(t + 1) * P, :])
                    pr = dst_pairs[t // 2]
                    rmsnorm_tile(pr[:, (t % 2) * E:(t % 2) * E + E], xg[:],
                                 dst2=(dst2[t] if dst2 and t < 4 else None))
                return
            for gi, g0 in enumerate(range(t0, t1, 4)):
                xg = xpool.tile([P, 4 * E], f16, tag="xing", name="xing")
                eng = nc.sync if (gi + qoff) % 2 == 0 else nc.gpsimd
                eng.dma_start(
                    out=xg.rearrange("p (t e) -> p t e", e=E),
                    in_=x_dram[g0 * P:(g0 + 4) * P, :].rearrange(
                        "(t p) e -> p t e", p=P))
                for t in range(4):
                    ti = g0 + t
                    pr = dst_pairs[ti // 2]
                    half = (ti % 2) * E
                    rmsnorm_tile(pr[:, half:half + E],
                                 xg[:, t * E:(t + 1) * E],
                                 dst2=(dst2[ti] if dst2 and ti < 4 else None),
                                 mul_eng=mul_eng)

        def transpose_local(srcs, dstT):
            for fc in range(4):
                ps = psp.tile([P, RPC], f32, tag="projps", bufs=4, name="trps")
                for t in range(4):
                    nc.tensor.transpose(ps[:, t * P:(t + 1) * P],
                                        srcs[t][:, fc * P:(fc + 1) * P]
                                        .bitcast(f32),
                                        ident[:])
                nc.scalar.copy(dstT[fc][:], ps[:])

        def aggregate(mi, lhs_pairs, aggpool, scale=1.0):
            """returns 4 feature-major f16 [128, 512] blocks of mat_mi @ x.

            fp8 DoubleRow: contraction over pairs of 128-node planes; lhsT
            free dims (2, 128 feats), rhs free dims (2, 512 rows)."""
            mst = ExitStack()
            matgp = mst.enter_context(tc.tile_pool(name=f"matg{mi}", bufs=2))
            pss = [psp.tile([P, E], f32, tag=f"agps{b}", name=f"agps{b}")
                   for b in range(4)]
            NPAIR = NT // 2
            for g in range(4):
                mt = matgp.tile([P, 4 * 2 * RPC], f8, tag="matg", name="matg")
                nc.sync.dma_start(
                    out=mt.rearrange("p (t two e) -> p t two e", e=RPC, two=2),
                    in_=mat_d[mi][g])
                for t in range(4):
                    pi = g * 4 + t
                    for b in range(4):
                        nc.tensor.matmul(
                            pss[b][:],
                            lhsT=bc(lhs_pairs[pi], [(E, 2), (1, P)], off=b * P),
                            rhs=bc(mt, [(RPC, 2), (1, RPC)], off=t * 2 * RPC),
                            start=(pi == 0), stop=(pi == NPAIR - 1),
                            perf_mode=DR)
            outt = []
            for b in range(4):
                at = aggpool.tile([P, E], f16, tag=f"ag{mi}_{b}",
                                  name=f"ag{mi}_{b}")
                nc.scalar.activation(out=at[:], in_=pss[b][:], func=AF.Copy,
                                     scale=scale)
                outt.append(at)
            mst.close()
            return outt

        def wload(m):
            """One merged q|k|v weight DMA for module m (12KB/partition).
            Issued on the Pool SWDGE queue so weights never queue behind
            the bulk x/mat stream on HWDGE."""
            wt = wpool.tile([P, 3 * 4 * E], f16, tag="wqkv", bufs=3,
                            name=f"wqkv{m}")
            nc.gpsimd.dma_start(
                out=wt.rearrange("p (k fc e) -> p k fc e", e=E, fc=4),
                in_=wqkv_d[m])
            return wt

        def module_proj(m, qsrc, ksrc, wt=None, pool_copies=False):
            if wt is None:
                wt = wload(m)
            q_sb, k_sb, v_sb = [], [], []
            for (srcT, ki, lst) in ((qsrc, 0, q_sb), (ksrc, 1, k_sb),
                                    (hTl, 2, v_sb)):
                for b in range(LT):
                    ps = psp.tile([P, E], f32, tag="projps", bufs=4,
                                  name="projps")
                    for fc in range(4):
                        nc.tensor.matmul(
                            ps[:],
                            lhsT=srcT[fc][:, b * P:(b + 1) * P],
                            rhs=wt[:, (ki * 4 + fc) * E:(ki * 4 + fc + 1) * E],
                            start=(fc == 0), stop=(fc == 3))
                    dt = qkvp.tile([P, E], f16, tag=f"qkv{ki}_{b}", bufs=2,
                                   name=f"qkv{b}")
                    dst = bc(dt, [(1, 8), (8, 64)]) if ki == 2 else dt[:]
                    if pool_copies and ki != 2:
                        nc.gpsimd.tensor_copy(out=dst, in_=ps[:])
                    else:
                        nc.scalar.copy(dst, ps[:])
                    lst.append(dt)
            return (q_sb, k_sb, v_sb)

        def module_sdpa(qkv, t, branch_att, first, offload=True):
                q_sb, k_sb, v_sb = qkv
                q_t, k_t, v_t = q_sb[t], k_sb[t], v_sb[t]
                tmp = tmpp.tile([P, H * H * D], f16, tag="sdpa", bufs=4,
                                name="sdpa")
                nc.vector.tensor_tensor(
                    out=bc(tmp, [(512, 8), (64, 8), (1, 64)]),
                    in0=bc(q_t, [(64, 8), (0, 8), (1, 64)]),
                    in1=bc(k_t, [(0, 8), (64, 8), (1, 64)]),
                    op=OP.mult)
                # first halving level on Pool for paired modules (Pool has
                # slack; DVE is critical). Solo modules keep the chain on
                # DVE to avoid cross-engine ping-pong latency.
                eng0 = nc.gpsimd if offload else nc.vector
                eng0.tensor_tensor(
                    out=bc(tmp, [(64, 64), (1, 32)]),
                    in0=bc(tmp, [(64, 64), (1, 32)]),
                    in1=bc(tmp, [(64, 64), (1, 32)], off=32),
                    op=OP.add)
                for dd in (16, 8, 4, 2):
                    nc.vector.tensor_tensor(
                        out=bc(tmp, [(64, 64), (1, dd)]),
                        in0=bc(tmp, [(64, 64), (1, dd)]),
                        in1=bc(tmp, [(64, 64), (1, dd)], off=dd),
                        op=OP.add)
                s_t = smp.tile([P, H * H], f32, tag="s", name="s")
                nc.vector.tensor_reduce(
                    out=s_t[:], in_=bc(tmp, [(64, 64), (1, 2)]),
                    axis=AX.X, op=OP.add)
                ex_t = smp.tile([P, H * H], f16, tag="ex", name="ex")
                nc.scalar.activation(out=ex_t[:], in_=s_t[:], func=AF.Exp)
                den = smp.tile([P, H], f32, tag="den", name="den")
                nc.vector.tensor_reduce(
                    out=den[:], in_=ex_t.rearrange("p (h g) -> p h g", g=H),
                    axis=AX.X, op=OP.add)
                rden = smp.tile([P, H], f32, tag="rden", name="rden")
                nc.vector.reciprocal(out=rden[:], in_=den[:])
                a_t = smp.tile([P, H * H], f16, tag="a", name="a")
                nc.vector.tensor_tensor(
                    out=bc(a_t, [(8, 8), (1, 8)]),
                    in0=bc(ex_t, [(8, 8), (1, 8)]),
                    in1=bc(rden, [(1, 8), (0, 8)]),
                    op=OP.mult)
                tmp2 = tmpp.tile([P, H * H * D], f16, tag="sdpa", bufs=4,
                                 name="sdpa2")
                nc.vector.tensor_tensor(
                    out=bc(tmp2, [(512, 8), (8, 64), (1, 8)]),
                    in0=bc(a_t, [(8, 8), (0, 64), (1, 8)]),
                    in1=bc(v_t, [(0, 8), (8, 64), (1, 8)]),
                    op=OP.mult)
                for gg in (4, 2):
                    nc.vector.tensor_tensor(
                        out=bc(tmp2, [(8, 512), (1, gg)]),
                        in0=bc(tmp2, [(8, 512), (1, gg)]),
                        in1=bc(tmp2, [(8, 512), (1, gg)], off=gg),
                        op=OP.add)
                if first:
                    nc.vector.tensor_tensor(
                        out=branch_att[t][:],
                        in0=bc(tmp2, [(8, 512)]),
                        in1=bc(tmp2, [(8, 512)], off=1),
                        op=OP.add)
                else:
                    # pair-sum on Pool when offloading (else DVE), final
                    # accumulate always on Pool
                    rt = smp.tile([P, E], f32, tag="avred", name="avred")
                    eng1 = nc.gpsimd if offload else nc.vector
                    eng1.tensor_tensor(
                        out=rt[:],
                        in0=bc(tmp2, [(8, 512)]),
                        in1=bc(tmp2, [(8, 512)], off=1),
                        op=OP.add)
                    nc.gpsimd.tensor_tensor(out=branch_att[t][:],
                                            in0=branch_att[t][:], in1=rt[:],
                                            op=OP.add)

        def module(m, qsrc, ksrc, branch_att, first, wt=None, offload=True):
            qkv = module_proj(m, qsrc, ksrc, wt)
            for t in range(LT):
                module_sdpa(qkv, t, branch_att, first, offload=offload)

        def ffn_load(w1_dram, w2_dram, b1_t):
            """FFN weight DMAs on the Pool SWDGE queue."""
            st = {"stack": ExitStack(), "b1": b1_t, "xn": [None] * LT}
            fsb = st["stack"].enter_context(tc.tile_pool(name="ffn_sb", bufs=1))
            w2_t = fsb.tile([P, 16 * E], f16, tag="w2", name="w2")
            nc.gpsimd.dma_start(
                out=w2_t.rearrange("p (fc e) -> p fc e", e=E),
                in_=w2_dram[:, :].rearrange("(fc p) e -> p fc e", p=P))
            w1_t = fsb.tile([P, 4 * FF], f16, tag="w1", name="w1")
            nc.gpsimd.dma_start(
                out=w1_t.rearrange("p (fc e) -> p fc e", e=FF),
                in_=w1_dram[:, :].rearrange("(fc p) e -> p fc e", p=P))
            st.update(fsb=fsb, w1=w1_t, w2=w2_t)
            return st

        def ffn_norm(st, branch_att, t):
            xt = st["fsb"].tile([P, E], f32, tag=f"fx{t}", name=f"fx{t}")
            rmsnorm_tile(xt[:], branch_att[t][:])
            st["xn"][t] = xt

        def ffn_tr(st):
            """Transpose normed tiles feature-major; pin layer-2 psums."""
            xn_tiles, xnT = st["xn"], []
            for fc in range(4):
                ps = psp.tile([P, RPC], f32, tag="projps", bufs=4,
                              name="ftr")
                for t in range(4):
                    nc.tensor.transpose(ps[:, t * P:(t + 1) * P],
                                        xn_tiles[t][:, fc * P:(fc + 1) * P],
                                        ident[:])
                xt = st["fsb"].tile([P, RPC], f16, tag=f"fxT{fc}",
                                    name=f"fxT{fc}")
                nc.scalar.copy(xt[:], ps[:])
                xnT.append(xt)
            st.update(xnT=xnT,
                      pss2=[psp.tile([P, E], f32, tag=f"agps{b}",
                                     name=f"fo{b}") for b in range(LT)])

        def ffn_start(branch_att, w1_dram, w2_dram, b1_t):
            st = ffn_load(w1_dram, w2_dram, b1_t)
            for t in range(LT):
                ffn_norm(st, branch_att, t)
            ffn_tr(st)
            return st

        def ffn_blocks(st, ffb0, ffb1):
            """Hidden blocks [ffb0, ffb1): layer1 -> gelu -> layer2 accum."""
            w1_t, w2_t, xnT, pss2 = st["w1"], st["w2"], st["xnT"], st["pss2"]
            for ffb in range(ffb0, ffb1):
                ps = psp.tile([P, RPC], f32, tag="projps", bufs=4,
                              name="fps1")
                for fc in range(4):
                    nc.tensor.matmul(
                        ps[:],
                        lhsT=w1_t[:, fc * FF + ffb * P:fc * FF + (ffb + 1) * P],
                        rhs=xnT[fc][:],
                        start=(fc == 0), stop=(fc == 3))
                gt = st["fsb"].tile([P, RPC], f16, tag=f"g1_{ffb % 4}",
                                    name=f"g1_{ffb}")
                nc.scalar.activation(out=gt[:], in_=ps[:], func=AF.Gelu,
                                     bias=st["b1"][:, ffb:ffb + 1], scale=1.0)
                for b in range(LT):
                    nc.tensor.matmul(
                        pss2[b][:],
                        lhsT=gt[:, b * P:(b + 1) * P],
                        rhs=w2_t[:, ffb * E:(ffb + 1) * E],
                        start=(ffb == 0), stop=False)

        def ffn_finish(st, b2_t, out_dram):
            pss2 = st["pss2"]
            ot = st["fsb"].tile([P, 4 * E], f32, tag="fo", name="fo")
            for b in range(LT):
                nc.tensor.matmul(pss2[b][:], lhsT=ones1[:], rhs=b2_t[:],
                                 start=False, stop=True)
                nc.scalar.copy(ot[:, b * E:(b + 1) * E], pss2[b][:])
                nc.sync.dma_start(
                    out=out_dram[b * P:(b + 1) * P, :],
                    in_=ot[:, b * E:(b + 1) * E])
            st["stack"].close()

        def ffn(branch_att, w1_dram, w2_dram, b1_t, b2_t, out_dram):
            st = ffn_start(branch_att, w1_dram, w2_dram, b1_t)
            ffn_blocks(st, 0, FF // P)
            ffn_finish(st, b2_t, out_dram)

        # ======== emission order (the schedule) ========
        # Pool open/close must be LIFO: agglate (aggs 1-4) and eearly
        # outlive hfull; agg0/erest nest inside.
        aggl12_stack = ExitStack()
        agglate12 = aggl12_stack.enter_context(
            tc.tile_pool(name="agglate12", bufs=1))
        aggl34_stack = ExitStack()
        agglate34 = aggl34_stack.enter_context(
            tc.tile_pool(name="agglate34", bufs=1))
        eearly_stack = ExitStack()
        eearly = eearly_stack.enter_context(tc.tile_pool(name="eearly", bufs=1))
        hstack = ExitStack()
        hfp = hstack.enter_context(tc.tile_pool(name="hfull", bufs=1))

        h_sb = [hfp.tile([P, 2 * E], f8, tag=f"h{t}", name=f"hsb{t}")
                for t in range(NT // 2)]
        e_sb = [eearly.tile([P, 2 * E], f8, tag=f"e{t}", name=f"esb{t}")
                for t in range(2)]
        x04_stack = ExitStack()
        x04p = x04_stack.enter_context(tc.tile_pool(name="x04", bufs=1))
        h04 = [x04p.tile([P, E], f32r, tag=f"h04_{t}", name=f"h04_{t}")
               for t in range(4)]
        e04 = [x04p.tile([P, E], f32r, tag=f"e04_{t}", name=f"e04_{t}")
               for t in range(4)]

        # weights for the two agg-free modules go first in the DMA queue
        wt1 = wload(1)
        wt5 = wload(5)
        load_norm(xe_d, e_sb, 0, 4, eearly, dst2=e04)
        load_norm(xn_d, h_sb, 0, 4, hfp, dst2=h04)
        transpose_local(e04, eTl)
        transpose_local(h04, hTl)
        x04_stack.close()

        # modules 1 and 5 need no aggregates (only hTl/eTl) - start DVE early
        module(1, eTl, eTl, xatt_h, first=True, wt=wt1, offload=False)
        module(5, eTl, eTl, xatt_e, first=True, wt=wt5, offload=False)

        load_norm(xn_d, h_sb, 4, NT, hfp)

        # interleave aggregate and projection emission so the PE stream
        # always runs ahead of the DVE (SDPA) stream
        agg0_stack = ExitStack()
        agg0pool = agg0_stack.enter_context(tc.tile_pool(name="agg0p", bufs=1))
        agg0 = aggregate(0, h_sb, agg0pool, scale=1.0 / 512)
        qkv0 = module_proj(0, agg0, hTl)
        agg1 = aggregate(1, h_sb, agglate12)
        for t in range(LT):
            module_sdpa(qkv0, t, xatt_h, first=False)
        qkv4 = module_proj(4, agg0, hTl)
        agg2 = aggregate(2, h_sb, agglate12)
        for t in range(LT):
            module_sdpa(qkv4, t, xatt_e, first=False)
        agg0_stack.close()
        hstack.close()

        qkv2 = module_proj(2, eTl, agg1)

        # rest of e
        erest_stack = ExitStack()
        erest = erest_stack.enter_context(tc.tile_pool(name="erest", bufs=1))
        e_sb += [erest.tile([P, 2 * E], f8, tag=f"e{t}", name=f"esb{t}")
                 for t in range(2, NT // 2)]
        load_norm(xe_d, e_sb, 4, NT, erest, mul_eng="act")
        for t in range(LT):
            module_sdpa(qkv2, t, xatt_h, first=False, offload=False)

        agg3 = aggregate(3, e_sb, agglate34)
        qkv3 = module_proj(3, hTl, agg3)
        agg4 = aggregate(4, e_sb, agglate34)
        for t in range(LT):
            module_sdpa(qkv3, t, xatt_h, first=False, offload=False)
        erest_stack.close()
        eearly_stack.close()

        # ---- tail: interleave ffn-h's PE blocks with m7/m6 SDPA (DVE) so
        # neither engine runs solo (PE executes in emission order).
        qkv7 = module_proj(7, hTl, agg4)
        fh = ffn_start(xatt_h, w1h_d, w2h_d, b1h_t)
        module_sdpa(qkv7, 0, xatt_e, False)
        ffn_blocks(fh, 0, 3)
        module_sdpa(qkv7, 1, xatt_e, False)
        ffn_blocks(fh, 3, 6)
        qkv6 = module_proj(6, eTl, agg2)
        module_sdpa(qkv7, 2, xatt_e, False)
        ffn_blocks(fh, 6, 9)
        module_sdpa(qkv7, 3, xatt_e, False)
        ffn_blocks(fh, 9, 11)
        module_sdpa(qkv6, 0, xatt_e, False)
        ffn_blocks(fh, 11, 13)
        module_sdpa(qkv6, 1, xatt_e, False)
        ffn_blocks(fh, 13, 16)
        ffn_finish(fh, b2h_t, outh_d)
        fe = ffn_load(w1e_d, w2e_d, b1e_t)
        module_sdpa(qkv6, 2, xatt_e, False)
        ffn_norm(fe, xatt_e, 0)
        ffn_norm(fe, xatt_e, 1)
        module_sdpa(qkv6, 3, xatt_e, False)
        ffn_norm(fe, xatt_e, 2)
        ffn_norm(fe, xatt_e, 3)
        ffn_tr(fe)
        ffn_blocks(fe, 0, FF // P)
        ffn_finish(fe, b2e_t, oute_d)
        aggl34_stack.close()
        aggl12_stack.close()

    _split_big_waits(nc, mybir)
    return nc


def _get_program():
    if "nc" not in _PROGRAM_CACHE:
        _PROGRAM_CACHE["nc"] = _build_program()
    return _PROGRAM_CACHE["nc"]


def _prep_inputs(x_node, x_edge, adj, Wq, Wk, Wv,
                 proj_he_h, proj_eh_h, proj_he_e, proj_eh_e,
                 rms1_h, rms1_e, rms2_h,
                 w1_h, b1_h, w2_h, b2_h, w1_e, b1_e, w2_e, b2_e):
    """Per-core input dicts. Weight folding + row rotation happen here.

    Aggregation mats are stored fp8(e4m3), pre-scaled so entries are O(1):
    adj x4096 (entries U[0, 1/4096]) and proj x8 (entries N(0, 1/64)).
    The descale is folded back into the downstream projection weights
    (q for modules 0/4 via agg0; k for modules 2/3/6/7 via aggs 1-4);
    for adj, 1/512 of the 1/4096 descale lives in the on-chip psum->f16
    copy (scale arg) so the folded f16 weights stay in normal range."""
    import ml_dtypes
    f = np.float32
    h16 = np.float16
    f8 = ml_dtypes.float8_e4m3
    wsrc_q = [rms1_h, rms1_e, rms1_e, rms1_h, rms1_h, rms1_e, rms1_e, rms1_h]
    wsrc_k = [rms1_h, rms1_e, rms1_h, rms1_e, rms1_h, rms1_e, rms1_h, rms1_e]
    qde = [1 / 8., 1, 1, 1, 1 / 8., 1, 1, 1]   # rest of adj descale (x 1/512 on-chip)
    kde = [1, 1, 1 / 8., 1 / 8., 1, 1, 1 / 8., 1 / 8.]  # proj descale
    wqT = np.stack([(Wq[m].T * wsrc_q[m][:, None]) * (0.125 * qde[m])
                    for m in range(H)])
    wkT = np.stack([Wk[m].T * wsrc_k[m][:, None] * kde[m] for m in range(H)])
    wvT = np.stack([Wv[m].T * rms1_h[:, None] for m in range(H)])
    # merged [H, p, 3(q/k/v), fc, e]: one contiguous 12KB line per partition
    wqkv = np.stack([wqT, wkT, wvT], axis=1).reshape(H, 3, 4, 128, E)
    wqkv = np.ascontiguousarray(wqkv.transpose(0, 3, 1, 2, 4), h16)
    w1hT = np.ascontiguousarray((w1_h * rms2_h[None, :]).T, dtype=h16)
    w1eT = np.ascontiguousarray((w1_e * rms2_h[None, :]).T, dtype=h16)
    w2hT = np.ascontiguousarray(w2_h.T, dtype=h16)
    w2eT = np.ascontiguousarray(w2_e.T, dtype=h16)
    mats = [adj * 4096., proj_eh_h * 8., proj_eh_e * 8.,
            proj_he_h * 8., proj_he_e * 8.]

    shared = dict(wqkv=wqkv,
                  w1hT=w1hT, w2hT=w2hT, w1eT=w1eT, w2eT=w2eT,
                  b1h=b1_h.astype(f), b2h=b2_h.astype(f),
                  b1e=b1_e.astype(f), b2e=b2_e.astype(f))
    in_maps = []
    for c in range(NCORES):
        r0 = c * RPC
        m = dict(shared)
        m["xn"] = np.ascontiguousarray(np.roll(x_node, -r0, axis=0), h16)
        m["xe"] = np.ascontiguousarray(np.roll(x_edge, -r0, axis=0), h16)
        for i, mat in enumerate(mats):
            mt = np.roll(np.ascontiguousarray(mat[r0:r0 + RPC].T),
                         -r0, axis=0).astype(f8)  # [N, RPC] node-major
            # [g, p, t, two, r]: node n = ((g*4 + t)*2 + two)*128 + p
            m[f"mat{i}"] = np.ascontiguousarray(
                mt.reshape(4, 4, 2, 128, RPC).transpose(0, 3, 1, 2, 4))
        in_maps.append(m)
    return in_maps


def kernel(**inputs):
    from concourse.bass_utils import run_bass_kernel_spmd
    nc = _get_program()
    in_maps = _prep_inputs(**{k: np.asarray(v) for k, v in inputs.items()})
    res = run_bass_kernel_spmd(nc, in_maps, list(range(NCORES))).results
    x_h = np.concatenate([res[c]["outh"] for c in range(NCORES)], axis=0)
    x_e = np.concatenate([res[c]["oute"] for c in range(NCORES)], axis=0)
    return (x_h, x_e)

